# revision 31
# baseline (speedup 1.0000x reference)
"""Trainium2 Bass kernel for causal self-attention with ALiBi + GQA.

Problem: B=2, T=2048, C=2048, 16 q-heads / 4 kv-heads, head_dim=128.
  q = x@q_w.T, k = x@k_w.T, v = x@v_w.T (GQA repeat 4x)
  att = softmax(q k^T/sqrt(d) + causal + alibi); out = (att v) @ o_w.T

Sharding over 8 NeuronCores: core c -> batch c//4, kv-group g=c%4
(q-heads 4g..4g+3, kv-head g).  Each core computes attention for its 4
heads on its batch plus a partial o-projection over its 512 channels;
the host sums the 4 partials per batch.

On-chip design (per core, all matmuls fp16, fp32 accumulate — fp16 is
1 cycle/row on the PE like bf16 but with 4x the mantissa; all values
here are bounded so there is no range risk):
  - x is host-cast to fp16 and host-TRANSPOSED (xT [C,T]) so projection
    moving operands load with plain contiguous DMA.
  - Projections make QT [d,t], KT [d,t] (transposed) and V natural
    [t,d] with a ones-column appended, so the AV matmul also emits the
    softmax denominator for free.
  - Scores are computed transposed sT[k,q] = KTblk.T @ QT (moving free
    dim 512, causally narrowed per diagonal offset); ALiBi + causal
    folded in via host-precomputed additive f32 tiles (DVE) and a
    per-(head,offset) bias in the ACT exp.  No max-subtraction needed:
    scores are small (~N(0,0.8)) and masked entries use -1e9.
  - y accumulates un-normalized; delayed normalization via per-row
    reciprocal of the ones-column sums, then PE-transpose -> yT feeds
    the o-projection (psum -> ACT/DVE copy -> fp16 -> DMA out; host
    sums the 4 partials per batch in fp32).
Measured: ~285 us/core on TRN2, L2 rel err ~6.3e-4 vs fp32 reference.
"""

import math
import sys
from contextlib import ExitStack

import numpy as np

sys.path.insert(0, "/opt/trn_rl_repo")

import ml_dtypes  # noqa: E402

import concourse.bacc as bacc  # noqa: E402
import concourse.bass as bass  # noqa: E402
import concourse.mybir as mybir  # noqa: E402
import concourse.tile as tile  # noqa: E402

F16 = mybir.dt.float16
F32 = mybir.dt.float32
NP_F16 = np.float16

B, T, C = 2, 2048, 2048
H, HKV, D = 16, 4, 128
P = 128
CH = 512                 # q-chunk (moving free dim)
NCB = C // P             # 16 contraction blocks
NTB = T // P             # 16 t-blocks
NCHK = T // CH           # 4 q-chunks
NQH = 4                  # local q heads per core
SCALE = 1.0 / math.sqrt(D)
MASK_NEG = -1.0e9


def _alibi_slopes(n):
    start = 2 ** (-(2 ** (-(math.log2(n) - 3))))
    return np.array([start * start**i for i in range(n)], dtype=np.float64)


def build_program():
    """Build the (SPMD-identical) single-core program."""
    nc = bacc.Bacc("TRN2", target_bir_lowering=False, debug=False, num_devices=8)

    xT_ap = nc.dram_tensor("xT", [C, T], F16, kind="ExternalInput").ap()
    qwT_ap = nc.dram_tensor("qwT", [P, NCB, NQH * P], F16, kind="ExternalInput").ap()
    kwT_ap = nc.dram_tensor("kwT", [P, NCB, D], F16, kind="ExternalInput").ap()
    vwT_ap = nc.dram_tensor("vwT", [P, NCB, D], F16, kind="ExternalInput").ap()
    owT_ap = nc.dram_tensor("owT", [P, NQH, C], F16, kind="ExternalInput").ap()
    mcl_ap = nc.dram_tensor("mcl", [P, NQH, CH], F32, kind="ExternalInput").ap()
    mcb_ap = nc.dram_tensor("mcb", [P, NQH, 4, CH], F32, kind="ExternalInput").ap()
    bias_ap = nc.dram_tensor("bias", [P, NQH, 16], F32, kind="ExternalInput").ap()
    id_ap = nc.dram_tensor("ident", [P, P], F16, kind="ExternalInput").ap()
    out_ap = nc.dram_tensor("out_p", [T, C], F16, kind="ExternalOutput").ap()

    EXP = mybir.ActivationFunctionType.Exp

    with tile.TileContext(nc) as tc, ExitStack() as ctx:
        const = ctx.enter_context(tc.tile_pool(name="const", bufs=1))
        qwT_sb = const.tile([P, NCB, NQH * P], F16, name="qwT_sb")
        kwT_sb = const.tile([P, NCB, D], F16, name="kwT_sb")
        vwT_sb = const.tile([P, NCB, D], F16, name="vwT_sb")
        owT_sb = const.tile([P, NQH, C], F16, name="owT_sb")
        mcl_sb = const.tile([P, NQH, CH], F32, name="mcl_sb")
        mcb_sb = const.tile([P, NQH, 4, CH], F32, name="mcb_sb")
        bias_sb = const.tile([P, NQH, 16], F32, name="bias_sb")
        id_sb = const.tile([P, P], F16, name="id_sb")

        QT_sb = const.tile([P, NQH, T], F16, name="QT_sb")
        KT_sb = const.tile([P, T], F16, name="KT_sb")
        Vaug_sb = const.tile([P, NTB, 132], F16, name="Vaug_sb")
        yT_sb = const.tile([P, NQH, T], F16, name="yT_sb")

        nc.sync.dma_start(qwT_sb[:], qwT_ap[:])
        nc.sync.dma_start(kwT_sb[:], kwT_ap[:])
        nc.sync.dma_start(vwT_sb[:], vwT_ap[:])

        nc.vector.memset(Vaug_sb[:, :, 128:129], 1.0)

        xT_pool = ctx.enter_context(tc.tile_pool(name="xT_pool", bufs=24))
        ps_pool = ctx.enter_context(tc.tile_pool(name="ps_pool", bufs=5, space="PSUM"))
        yps_pool = ctx.enter_context(tc.tile_pool(name="yps_pool", bufs=3, space="PSUM"))
        tp_pool = yps_pool
        t1_pool = ctx.enter_context(tc.tile_pool(name="t1_pool", bufs=8))
        exp_pool = ctx.enter_context(tc.tile_pool(name="exp_pool", bufs=22))
        oev_pool = ctx.enter_context(tc.tile_pool(name="oev_pool", bufs=4))
        yn_pool = ctx.enter_context(tc.tile_pool(name="yn_pool", bufs=4))
        rc_pool = ctx.enter_context(tc.tile_pool(name="rc_pool", bufs=4))

        # ---- Fused per-chunk pipeline: project chunk j, then attention for
        # chunk j (legal because causality means queries in chunk j only
        # attend to keys/values t <= chunk j), then its o-projection.
        # This overlaps PE-heavy projections of chunk j+1 with the
        # DVE/ACT-heavy softmax chain of chunk j. ----
        def project_chunk(j):
            t0 = j * CH
            xts = []
            for cb in range(NCB):
                xt = xT_pool.tile([P, CH], F16, name=f"xt_{j}_{cb}", tag="xt")
                nc.sync.dma_start(xt[:], xT_ap[cb * P : (cb + 1) * P, t0 : t0 + CH])
                xts.append(xt)
            for qh in range(NQH):
                ps = ps_pool.tile([P, CH], F32, name=f"psq_{j}_{qh}", tag="ps")
                for cb in range(NCB):
                    nc.tensor.matmul(
                        ps[:],
                        lhsT=qwT_sb[:, cb, qh * P : (qh + 1) * P],
                        rhs=xts[cb][:],
                        start=(cb == 0),
                        stop=(cb == NCB - 1),
                    )
                nc.vector.tensor_copy(QT_sb[:, qh, t0 : t0 + CH], ps[:])
            psk = ps_pool.tile([P, CH], F32, name=f"psk_{j}", tag="ps")
            for cb in range(NCB):
                nc.tensor.matmul(
                    psk[:],
                    lhsT=kwT_sb[:, cb, :],
                    rhs=xts[cb][:],
                    start=(cb == 0),
                    stop=(cb == NCB - 1),
                )
            nc.vector.tensor_copy(KT_sb[:, t0 : t0 + CH], psk[:])
            for tb in range(CH // P):
                gtb = j * (CH // P) + tb
                psv = ps_pool.tile([P, P], F32, name=f"psv_{j}_{tb}", tag="ps")
                for cb in range(NCB):
                    nc.tensor.matmul(
                        psv[:],
                        lhsT=xts[cb][:, tb * P : (tb + 1) * P],
                        rhs=vwT_sb[:, cb, :],
                        start=(cb == 0),
                        stop=(cb == NCB - 1),
                    )
                nc.vector.tensor_copy(Vaug_sb[:, gtb, 0:128], psv[:])

        def oproj_tblock(tb):
            for nch in range(C // CH):
                pso = ps_pool.tile([P, CH], F32, name=f"pso_{tb}_{nch}", tag="ps")
                for hb in range(NQH):
                    nc.tensor.matmul(
                        pso[:],
                        lhsT=yT_sb[:, hb, tb * P : (tb + 1) * P],
                        rhs=owT_sb[:, hb, nch * CH : (nch + 1) * CH],
                        start=(hb == 0),
                        stop=(hb == NQH - 1),
                    )
                ot = oev_pool.tile([P, CH], F16, name=f"ot_{tb}_{nch}", tag="ot")
                if (tb + nch) % 2 == 0:
                    nc.scalar.copy(ot[:], pso[:])
                else:
                    nc.vector.tensor_copy(ot[:], pso[:])
                nc.sync.dma_start(
                    out_ap[tb * P : (tb + 1) * P, nch * CH : (nch + 1) * CH], ot[:]
                )

        def attention_chunk(j):
            q0 = j * CH
            nkb = 4 * j + 4
            for h in range(NQH):
                ets = []
                for kb in range(nkb):
                    oi = kb - 4 * j
                    # q-columns below oi*P are fully causal-masked; skip them
                    qoff = oi * P if oi > 0 else 0
                    pss = ps_pool.tile([P, CH], F32, name=f"pss_{h}_{j}_{kb}", tag="ps")
                    nc.tensor.matmul(
                        pss[:, qoff:],
                        lhsT=KT_sb[:, kb * P : (kb + 1) * P],
                        rhs=QT_sb[:, h, q0 + qoff : q0 + CH],
                        start=True,
                        stop=True,
                    )
                    t1 = t1_pool.tile([P, CH], F32, name=f"t1_{h}_{j}_{kb}", tag="t1")
                    if oi >= 0:
                        nc.vector.tensor_add(
                            t1[:, qoff:], pss[:, qoff:], mcb_sb[:, h, oi, qoff:]
                        )
                    else:
                        nc.vector.tensor_add(t1[:], pss[:], mcl_sb[:, h, :])
                    et = exp_pool.tile([P, CH], F16, name=f"et_{h}_{j}_{kb}", tag="et")
                    oidx = oi + 12
                    nc.scalar.activation(
                        et[:, qoff:],
                        t1[:, qoff:],
                        EXP,
                        bias=bias_sb[:, h, oidx : oidx + 1],
                        scale=SCALE,
                    )
                    ets.append(et)
                for qb in range(CH // P):
                    gqb = j * (CH // P) + qb
                    yps = yps_pool.tile([P, 132], F32, name=f"yps_{h}_{gqb}", tag="yps")
                    for kb in range(gqb + 1):
                        nc.tensor.matmul(
                            yps[:, 0:129],
                            lhsT=ets[kb][:, qb * P : (qb + 1) * P],
                            rhs=Vaug_sb[:, kb, 0:129],
                            start=(kb == 0),
                            stop=(kb == gqb),
                        )
                    recip = rc_pool.tile([P, 1], F32, name=f"rc_{h}_{gqb}", tag="rc")
                    nc.vector.reciprocal(recip[:], yps[:, 128:129])
                    yn = yn_pool.tile([P, P], F16, name=f"yn_{h}_{gqb}", tag="yn")
                    nc.vector.tensor_scalar_mul(yn[:], yps[:, 0:128], recip[:])
                    tp = tp_pool.tile([P, P], F16, name=f"tp_{h}_{gqb}", tag="yps")
                    nc.tensor.transpose(tp[:], yn[:], id_sb[:])
                    nc.vector.tensor_copy(yT_sb[:, h, gqb * P : (gqb + 1) * P], tp[:])

        for j in range(NCHK):
            project_chunk(j)
            if j == 0:
                # constants first needed by attention/o-projection; issued
                # after chunk-0's projection DMAs so those aren't delayed
                nc.sync.dma_start(owT_sb[:], owT_ap[:])
                nc.sync.dma_start(mcl_sb[:], mcl_ap[:])
                nc.sync.dma_start(mcb_sb[:], mcb_ap[:])
                nc.sync.dma_start(bias_sb[:], bias_ap[:])
                nc.sync.dma_start(id_sb[:], id_ap[:])
            attention_chunk(j)
            for tb in range(j * (CH // P), (j + 1) * (CH // P)):
                oproj_tblock(tb)

    nc.compile()
    return nc


def make_in_maps(x, q_w, k_w, v_w, o_w):
    """Host-side sharding/preprocessing -> per-core input dicts."""
    slopes = _alibi_slopes(H)
    x_bf = np.asarray(x, dtype=NP_F16)

    ident = np.eye(P, dtype=NP_F16)

    pi = np.arange(P, dtype=np.float32)[:, None]
    mj = np.arange(CH, dtype=np.float32)[None, :]

    in_maps = []
    for c in range(8):
        b, g = c // 4, c % 4
        qsl = slice(4 * g * P, (4 * g + 4) * P)
        ksl = slice(g * P, (g + 1) * P)

        qwT = np.ascontiguousarray(
            np.asarray(q_w[qsl].T, dtype=NP_F16).reshape(NCB, P, NQH * P).transpose(1, 0, 2)
        )
        kwT = np.ascontiguousarray(
            np.asarray(k_w[ksl].T, dtype=NP_F16).reshape(NCB, P, D).transpose(1, 0, 2)
        )
        vwT = np.ascontiguousarray(
            np.asarray(v_w[ksl].T, dtype=NP_F16).reshape(NCB, P, D).transpose(1, 0, 2)
        )
        owT = np.ascontiguousarray(
            np.asarray(o_w[:, qsl].T, dtype=NP_F16).reshape(NQH, P, C).transpose(1, 0, 2)
        )

        mcl = np.empty((P, NQH, CH), dtype=np.float32)
        mcb = np.empty((P, NQH, 4, CH), dtype=np.float32)
        bias = np.empty((P, NQH, 16), dtype=np.float32)
        for h in range(NQH):
            sl = np.float32(slopes[4 * g + h])
            mcl[:, h, :] = (sl / np.float32(SCALE)) * (pi - mj)
            for oi in range(4):
                mcb[:, h, oi, :] = np.where(
                    oi * P + pi - mj > 0.0, np.float32(MASK_NEG), mcl[:, h, :]
                )
            for oidx in range(16):
                bias[:, h, oidx] = sl * np.float32(P * (oidx - 12))

        in_maps.append(
            dict(
                xT=np.ascontiguousarray(x_bf[b].T),
                qwT=qwT,
                kwT=kwT,
                vwT=vwT,
                owT=owT,
                mcl=mcl,
                mcb=mcb,
                bias=bias,
                ident=ident,
            )
        )
    return in_maps


def gather_output(results):
    out = np.zeros((B, T, C), dtype=np.float32)
    for c in range(8):
        out[c // 4] += results[c]["out_p"].astype(np.float32)
    return out


_NC_CACHE = {}


def get_program():
    if "nc" not in _NC_CACHE:
        _NC_CACHE["nc"] = build_program()
    return _NC_CACHE["nc"]


def kernel(x, q_w, k_w, v_w, o_w):
    from concourse.bass_utils import run_bass_kernel_spmd

    nc = get_program()
    in_maps = make_in_maps(x, q_w, k_w, v_w, o_w)
    res = run_bass_kernel_spmd(nc, in_maps, list(range(8)))
    return gather_output(res.results)


# revision 33
# speedup vs baseline: 1.0241x; 1.0241x over previous
"""Trainium2 Bass kernel for causal self-attention with ALiBi + GQA.

Problem: B=2, T=2048, C=2048, 16 q-heads / 4 kv-heads, head_dim=128.
  q = x@q_w.T, k = x@k_w.T, v = x@v_w.T (GQA repeat 4x)
  att = softmax(q k^T/sqrt(d) + causal + alibi); out = (att v) @ o_w.T

Sharding over 8 NeuronCores: core c -> batch c//4, kv-group g=c%4
(q-heads 4g..4g+3, kv-head g).  Each core computes attention for its 4
heads on its batch plus a partial o-projection over its 512 channels;
the host sums the 4 partials per batch.

On-chip design (per core, all matmuls fp16, fp32 accumulate — fp16 is
1 cycle/row on the PE like bf16 but with 4x the mantissa; all values
here are bounded so there is no range risk):
  - x is host-cast to fp16 and host-TRANSPOSED (xT [C,T]) so projection
    moving operands load with plain contiguous DMA.
  - Projections make QT [d,t], KT [d,t] (transposed) and V natural
    [t,d] with a ones-column appended, so the AV matmul also emits the
    softmax denominator for free.
  - Scores are computed transposed sT[k,q] = KTblk.T @ QT (moving free
    dim 512, causally narrowed per diagonal offset); ALiBi + causal
    folded in via host-precomputed additive f32 tiles (DVE) and a
    per-(head,offset) bias in the ACT exp.  No max-subtraction needed:
    scores are small (~N(0,0.8)) and masked entries use -1e9.
  - y accumulates un-normalized; delayed normalization via per-row
    reciprocal of the ones-column sums, then PE-transpose -> yT feeds
    the o-projection (psum -> ACT/DVE copy -> fp16 -> DMA out; host
    sums the 4 partials per batch in fp32).
Measured: ~285 us/core on TRN2, L2 rel err ~6.3e-4 vs fp32 reference.
"""

import math
import sys
from contextlib import ExitStack

import numpy as np

sys.path.insert(0, "/opt/trn_rl_repo")

import ml_dtypes  # noqa: E402

import concourse.bacc as bacc  # noqa: E402
import concourse.bass as bass  # noqa: E402
import concourse.mybir as mybir  # noqa: E402
import concourse.tile as tile  # noqa: E402

F16 = mybir.dt.float16
F32 = mybir.dt.float32
NP_F16 = np.float16

B, T, C = 2, 2048, 2048
H, HKV, D = 16, 4, 128
P = 128
CH = 512                 # q-chunk (moving free dim)
NCB = C // P             # 16 contraction blocks
NTB = T // P             # 16 t-blocks
NCHK = T // CH           # 4 q-chunks
NQH = 4                  # local q heads per core
SCALE = 1.0 / math.sqrt(D)
MASK_NEG = -1.0e9


def _alibi_slopes(n):
    start = 2 ** (-(2 ** (-(math.log2(n) - 3))))
    return np.array([start * start**i for i in range(n)], dtype=np.float64)


def build_program():
    """Build the (SPMD-identical) single-core program."""
    nc = bacc.Bacc("TRN2", target_bir_lowering=False, debug=False, num_devices=8)

    xT_ap = nc.dram_tensor("xT", [C, T], F16, kind="ExternalInput").ap()
    qwT_ap = nc.dram_tensor("qwT", [P, NCB, NQH * P], F16, kind="ExternalInput").ap()
    kwT_ap = nc.dram_tensor("kwT", [P, NCB, D], F16, kind="ExternalInput").ap()
    vwT_ap = nc.dram_tensor("vwT", [P, NCB, D], F16, kind="ExternalInput").ap()
    owT_ap = nc.dram_tensor("owT", [P, NQH, C], F16, kind="ExternalInput").ap()
    mcl_ap = nc.dram_tensor("mcl", [P, NQH, CH], F32, kind="ExternalInput").ap()
    mcb_ap = nc.dram_tensor("mcb", [P, NQH, 4, CH], F32, kind="ExternalInput").ap()
    bias_ap = nc.dram_tensor("bias", [P, NQH, 16], F32, kind="ExternalInput").ap()
    id_ap = nc.dram_tensor("ident", [P, P], F16, kind="ExternalInput").ap()
    out_ap = nc.dram_tensor("out_p", [T, C], F16, kind="ExternalOutput").ap()

    EXP = mybir.ActivationFunctionType.Exp

    with tile.TileContext(nc) as tc, ExitStack() as ctx:
        const = ctx.enter_context(tc.tile_pool(name="const", bufs=1))
        qwT_sb = const.tile([P, NCB, NQH * P], F16, name="qwT_sb")
        kwT_sb = const.tile([P, NCB, D], F16, name="kwT_sb")
        vwT_sb = const.tile([P, NCB, D], F16, name="vwT_sb")
        owT_sb = const.tile([P, NQH, C], F16, name="owT_sb")
        mcl_sb = const.tile([P, NQH, CH], F32, name="mcl_sb")
        mcb_sb = const.tile([P, NQH, 4, CH], F32, name="mcb_sb")
        bias_sb = const.tile([P, NQH, 16], F32, name="bias_sb")
        id_sb = const.tile([P, P], F16, name="id_sb")

        QT_sb = const.tile([P, NQH, T], F16, name="QT_sb")
        KT_sb = const.tile([P, T], F16, name="KT_sb")
        Vaug_sb = const.tile([P, NTB, 132], F16, name="Vaug_sb")
        yT_sb = const.tile([P, NQH, T], F16, name="yT_sb")

        nc.sync.dma_start(qwT_sb[:], qwT_ap[:])
        nc.sync.dma_start(kwT_sb[:], kwT_ap[:])
        nc.sync.dma_start(vwT_sb[:], vwT_ap[:])

        nc.vector.memset(Vaug_sb[:, :, 128:129], 1.0)

        xT_pool = ctx.enter_context(tc.tile_pool(name="xT_pool", bufs=24))
        ps_pool = ctx.enter_context(tc.tile_pool(name="ps_pool", bufs=5, space="PSUM"))
        yps_pool = ctx.enter_context(tc.tile_pool(name="yps_pool", bufs=2, space="PSUM"))
        tp_pool = ctx.enter_context(tc.tile_pool(name="tp_pool", bufs=1, space="PSUM"))
        t1_pool = ctx.enter_context(tc.tile_pool(name="t1_pool", bufs=8))
        exp_pool = ctx.enter_context(tc.tile_pool(name="exp_pool", bufs=22))
        oev_pool = ctx.enter_context(tc.tile_pool(name="oev_pool", bufs=4))
        yn_pool = ctx.enter_context(tc.tile_pool(name="yn_pool", bufs=4))
        rc_pool = ctx.enter_context(tc.tile_pool(name="rc_pool", bufs=4))

        # ---- Fused per-chunk pipeline: project chunk j, then attention for
        # chunk j (legal because causality means queries in chunk j only
        # attend to keys/values t <= chunk j), then its o-projection.
        # This overlaps PE-heavy projections of chunk j+1 with the
        # DVE/ACT-heavy softmax chain of chunk j. ----
        def project_chunk(j):
            t0 = j * CH
            xts = []
            for cb in range(NCB):
                xt = xT_pool.tile([P, CH], F16, name=f"xt_{j}_{cb}", tag="xt")
                nc.sync.dma_start(xt[:], xT_ap[cb * P : (cb + 1) * P, t0 : t0 + CH])
                xts.append(xt)
            for qh in range(NQH):
                ps = ps_pool.tile([P, CH], F32, name=f"psq_{j}_{qh}", tag="ps")
                for cb in range(NCB):
                    nc.tensor.matmul(
                        ps[:],
                        lhsT=qwT_sb[:, cb, qh * P : (qh + 1) * P],
                        rhs=xts[cb][:],
                        start=(cb == 0),
                        stop=(cb == NCB - 1),
                    )
                nc.vector.tensor_copy(QT_sb[:, qh, t0 : t0 + CH], ps[:])
            psk = ps_pool.tile([P, CH], F32, name=f"psk_{j}", tag="ps")
            for cb in range(NCB):
                nc.tensor.matmul(
                    psk[:],
                    lhsT=kwT_sb[:, cb, :],
                    rhs=xts[cb][:],
                    start=(cb == 0),
                    stop=(cb == NCB - 1),
                )
            nc.vector.tensor_copy(KT_sb[:, t0 : t0 + CH], psk[:])
            for tb in range(CH // P):
                gtb = j * (CH // P) + tb
                psv = ps_pool.tile([P, P], F32, name=f"psv_{j}_{tb}", tag="ps")
                for cb in range(NCB):
                    nc.tensor.matmul(
                        psv[:],
                        lhsT=xts[cb][:, tb * P : (tb + 1) * P],
                        rhs=vwT_sb[:, cb, :],
                        start=(cb == 0),
                        stop=(cb == NCB - 1),
                    )
                nc.vector.tensor_copy(Vaug_sb[:, gtb, 0:128], psv[:])

        def oproj_tblock(tb):
            for nch in range(C // CH):
                pso = ps_pool.tile([P, CH], F32, name=f"pso_{tb}_{nch}", tag="ps")
                for hb in range(NQH):
                    nc.tensor.matmul(
                        pso[:],
                        lhsT=yT_sb[:, hb, tb * P : (tb + 1) * P],
                        rhs=owT_sb[:, hb, nch * CH : (nch + 1) * CH],
                        start=(hb == 0),
                        stop=(hb == NQH - 1),
                    )
                ot = oev_pool.tile([P, CH], F16, name=f"ot_{tb}_{nch}", tag="ot")
                if (tb + nch) % 2 == 0:
                    nc.scalar.copy(ot[:], pso[:])
                else:
                    nc.vector.tensor_copy(ot[:], pso[:])
                nc.sync.dma_start(
                    out_ap[tb * P : (tb + 1) * P, nch * CH : (nch + 1) * CH], ot[:]
                )

        def attention_chunk(j):
            q0 = j * CH
            nkb = 4 * j + 4
            for h in range(NQH):
                ets = []
                for kb in range(nkb):
                    oi = kb - 4 * j
                    # q-columns below oi*P are fully causal-masked; skip them
                    qoff = oi * P if oi > 0 else 0
                    pss = ps_pool.tile([P, CH], F32, name=f"pss_{h}_{j}_{kb}", tag="ps")
                    nc.tensor.matmul(
                        pss[:, qoff:],
                        lhsT=KT_sb[:, kb * P : (kb + 1) * P],
                        rhs=QT_sb[:, h, q0 + qoff : q0 + CH],
                        start=True,
                        stop=True,
                    )
                    t1 = t1_pool.tile([P, CH], F32, name=f"t1_{h}_{j}_{kb}", tag="t1")
                    if oi >= 0:
                        nc.vector.tensor_add(
                            t1[:, qoff:], pss[:, qoff:], mcb_sb[:, h, oi, qoff:]
                        )
                    else:
                        nc.vector.tensor_add(t1[:], pss[:], mcl_sb[:, h, :])
                    et = exp_pool.tile([P, CH], F16, name=f"et_{h}_{j}_{kb}", tag="et")
                    oidx = oi + 12
                    nc.scalar.activation(
                        et[:, qoff:],
                        t1[:, qoff:],
                        EXP,
                        bias=bias_sb[:, h, oidx : oidx + 1],
                        scale=SCALE,
                    )
                    ets.append(et)
                for qb in range(CH // P):
                    gqb = j * (CH // P) + qb
                    yps = yps_pool.tile([P, 132], F32, name=f"yps_{h}_{gqb}", tag="yps")
                    for kb in range(gqb + 1):
                        nc.tensor.matmul(
                            yps[:, 0:129],
                            lhsT=ets[kb][:, qb * P : (qb + 1) * P],
                            rhs=Vaug_sb[:, kb, 0:129],
                            start=(kb == 0),
                            stop=(kb == gqb),
                        )
                    recip = rc_pool.tile([P, 1], F32, name=f"rc_{h}_{gqb}", tag="rc")
                    nc.vector.reciprocal(recip[:], yps[:, 128:129])
                    yn = yn_pool.tile([P, P], F16, name=f"yn_{h}_{gqb}", tag="yn")
                    nc.vector.tensor_scalar_mul(yn[:], yps[:, 0:128], recip[:])
                    tp = tp_pool.tile([P, P], F16, name=f"tp_{h}_{gqb}", tag="tp")
                    nc.tensor.transpose(tp[:], yn[:], id_sb[:])
                    nc.vector.tensor_copy(yT_sb[:, h, gqb * P : (gqb + 1) * P], tp[:])

        for j in range(NCHK):
            project_chunk(j)
            if j == 0:
                # constants first needed by attention/o-projection; issued
                # after chunk-0's projection DMAs so those aren't delayed
                nc.sync.dma_start(owT_sb[:], owT_ap[:])
                nc.sync.dma_start(mcl_sb[:], mcl_ap[:])
                nc.sync.dma_start(mcb_sb[:], mcb_ap[:])
                nc.sync.dma_start(bias_sb[:], bias_ap[:])
                nc.sync.dma_start(id_sb[:], id_ap[:])
            attention_chunk(j)
            for tb in range(j * (CH // P), (j + 1) * (CH // P)):
                oproj_tblock(tb)

    nc.compile()
    return nc


def make_in_maps(x, q_w, k_w, v_w, o_w):
    """Host-side sharding/preprocessing -> per-core input dicts."""
    slopes = _alibi_slopes(H)
    x_bf = np.asarray(x, dtype=NP_F16)

    ident = np.eye(P, dtype=NP_F16)

    pi = np.arange(P, dtype=np.float32)[:, None]
    mj = np.arange(CH, dtype=np.float32)[None, :]

    in_maps = []
    for c in range(8):
        b, g = c // 4, c % 4
        qsl = slice(4 * g * P, (4 * g + 4) * P)
        ksl = slice(g * P, (g + 1) * P)

        qwT = np.ascontiguousarray(
            np.asarray(q_w[qsl].T, dtype=NP_F16).reshape(NCB, P, NQH * P).transpose(1, 0, 2)
        )
        kwT = np.ascontiguousarray(
            np.asarray(k_w[ksl].T, dtype=NP_F16).reshape(NCB, P, D).transpose(1, 0, 2)
        )
        vwT = np.ascontiguousarray(
            np.asarray(v_w[ksl].T, dtype=NP_F16).reshape(NCB, P, D).transpose(1, 0, 2)
        )
        owT = np.ascontiguousarray(
            np.asarray(o_w[:, qsl].T, dtype=NP_F16).reshape(NQH, P, C).transpose(1, 0, 2)
        )

        mcl = np.empty((P, NQH, CH), dtype=np.float32)
        mcb = np.empty((P, NQH, 4, CH), dtype=np.float32)
        bias = np.empty((P, NQH, 16), dtype=np.float32)
        for h in range(NQH):
            sl = np.float32(slopes[4 * g + h])
            mcl[:, h, :] = (sl / np.float32(SCALE)) * (pi - mj)
            for oi in range(4):
                mcb[:, h, oi, :] = np.where(
                    oi * P + pi - mj > 0.0, np.float32(MASK_NEG), mcl[:, h, :]
                )
            for oidx in range(16):
                bias[:, h, oidx] = sl * np.float32(P * (oidx - 12))

        in_maps.append(
            dict(
                xT=np.ascontiguousarray(x_bf[b].T),
                qwT=qwT,
                kwT=kwT,
                vwT=vwT,
                owT=owT,
                mcl=mcl,
                mcb=mcb,
                bias=bias,
                ident=ident,
            )
        )
    return in_maps


def gather_output(results):
    out = np.zeros((B, T, C), dtype=np.float32)
    for c in range(8):
        out[c // 4] += results[c]["out_p"].astype(np.float32)
    return out


_NC_CACHE = {}


def get_program():
    if "nc" not in _NC_CACHE:
        _NC_CACHE["nc"] = build_program()
    return _NC_CACHE["nc"]


def kernel(x, q_w, k_w, v_w, o_w):
    from concourse.bass_utils import run_bass_kernel_spmd

    nc = get_program()
    in_maps = make_in_maps(x, q_w, k_w, v_w, o_w)
    res = run_bass_kernel_spmd(nc, in_maps, list(range(8)))
    return gather_output(res.results)


# revision 34
# speedup vs baseline: 1.0437x; 1.0191x over previous
"""Trainium2 Bass kernel for causal self-attention with ALiBi + GQA.

Problem: B=2, T=2048, C=2048, 16 q-heads / 4 kv-heads, head_dim=128.
  q = x@q_w.T, k = x@k_w.T, v = x@v_w.T (GQA repeat 4x)
  att = softmax(q k^T/sqrt(d) + causal + alibi); out = (att v) @ o_w.T

Sharding over 8 NeuronCores: core c -> batch c//4, kv-group g=c%4
(q-heads 4g..4g+3, kv-head g).  Each core computes attention for its 4
heads on its batch plus a partial o-projection over its 512 channels;
the host sums the 4 partials per batch.

On-chip design (per core, all matmuls fp16, fp32 accumulate — fp16 is
1 cycle/row on the PE like bf16 but with 4x the mantissa; all values
here are bounded so there is no range risk):
  - x is host-cast to fp16 and host-TRANSPOSED (xT [C,T]) so projection
    moving operands load with plain contiguous DMA.
  - Projections make QT [d,t], KT [d,t] (transposed) and V natural
    [t,d] with a ones-column appended, so the AV matmul also emits the
    softmax denominator for free.
  - Scores are computed transposed sT[k,q] = KTblk.T @ QT (moving free
    dim 512, causally narrowed per diagonal offset); ALiBi + causal
    folded in via host-precomputed additive f32 tiles (DVE) and a
    per-(head,offset) bias in the ACT exp.  No max-subtraction needed:
    scores are small (~N(0,0.8)) and masked entries use -1e9.
  - y accumulates un-normalized; delayed normalization via per-row
    reciprocal of the ones-column sums, then PE-transpose -> yT feeds
    the o-projection (psum -> ACT/DVE copy -> fp16 -> DMA out; host
    sums the 4 partials per batch in fp32).
Measured: ~285 us/core on TRN2, L2 rel err ~6.3e-4 vs fp32 reference.
"""

import math
import sys
from contextlib import ExitStack

import numpy as np

sys.path.insert(0, "/opt/trn_rl_repo")

import ml_dtypes  # noqa: E402

import concourse.bacc as bacc  # noqa: E402
import concourse.bass as bass  # noqa: E402
import concourse.mybir as mybir  # noqa: E402
import concourse.tile as tile  # noqa: E402

F16 = mybir.dt.float16
F32 = mybir.dt.float32
NP_F16 = np.float16

B, T, C = 2, 2048, 2048
H, HKV, D = 16, 4, 128
P = 128
CH = 512                 # q-chunk (moving free dim)
NCB = C // P             # 16 contraction blocks
NTB = T // P             # 16 t-blocks
NCHK = T // CH           # 4 q-chunks
NQH = 4                  # local q heads per core
SCALE = 1.0 / math.sqrt(D)
MASK_NEG = -1.0e9


def _alibi_slopes(n):
    start = 2 ** (-(2 ** (-(math.log2(n) - 3))))
    return np.array([start * start**i for i in range(n)], dtype=np.float64)


def build_program():
    """Build the (SPMD-identical) single-core program."""
    nc = bacc.Bacc("TRN2", target_bir_lowering=False, debug=False, num_devices=8)

    xT_ap = nc.dram_tensor("xT", [C, T], F16, kind="ExternalInput").ap()
    qwT_ap = nc.dram_tensor("qwT", [P, NCB, NQH * P], F16, kind="ExternalInput").ap()
    kwT_ap = nc.dram_tensor("kwT", [P, NCB, D], F16, kind="ExternalInput").ap()
    vwT_ap = nc.dram_tensor("vwT", [P, NCB, D], F16, kind="ExternalInput").ap()
    owT_ap = nc.dram_tensor("owT", [P, NQH, C], F16, kind="ExternalInput").ap()
    mcl_ap = nc.dram_tensor("mcl", [P, NQH, CH], F32, kind="ExternalInput").ap()
    mcb_ap = nc.dram_tensor("mcb", [P, NQH, 4, CH], F32, kind="ExternalInput").ap()
    bias_ap = nc.dram_tensor("bias", [P, NQH, 16], F32, kind="ExternalInput").ap()
    id_ap = nc.dram_tensor("ident", [P, P], F16, kind="ExternalInput").ap()
    out_ap = nc.dram_tensor("out_p", [T, C], F16, kind="ExternalOutput").ap()

    EXP = mybir.ActivationFunctionType.Exp

    with tile.TileContext(nc) as tc, ExitStack() as ctx:
        const = ctx.enter_context(tc.tile_pool(name="const", bufs=1))
        qwT_sb = const.tile([P, NCB, NQH * P], F16, name="qwT_sb")
        kwT_sb = const.tile([P, NCB, D], F16, name="kwT_sb")
        vwT_sb = const.tile([P, NCB, D], F16, name="vwT_sb")
        owT_sb = const.tile([P, NQH, C], F16, name="owT_sb")
        mcl_sb = const.tile([P, NQH, CH], F32, name="mcl_sb")
        mcb_sb = const.tile([P, NQH, 4, CH], F32, name="mcb_sb")
        bias_sb = const.tile([P, NQH, 16], F32, name="bias_sb")
        id_sb = const.tile([P, P], F16, name="id_sb")

        QT_sb = const.tile([P, NQH, T], F16, name="QT_sb")
        KT_sb = const.tile([P, T], F16, name="KT_sb")
        Vaug_sb = const.tile([P, NTB, 132], F16, name="Vaug_sb")
        yT_sb = const.tile([P, NQH, T], F16, name="yT_sb")

        nc.sync.dma_start(qwT_sb[:], qwT_ap[:])
        nc.sync.dma_start(kwT_sb[:], kwT_ap[:])
        nc.sync.dma_start(vwT_sb[:], vwT_ap[:])

        nc.vector.memset(Vaug_sb[:, :, 128:129], 1.0)

        xT_pool = ctx.enter_context(tc.tile_pool(name="xT_pool", bufs=24))
        ps_pool = ctx.enter_context(tc.tile_pool(name="ps_pool", bufs=5, space="PSUM"))
        yps_pool = ctx.enter_context(tc.tile_pool(name="yps_pool", bufs=2, space="PSUM"))
        tp_pool = ctx.enter_context(tc.tile_pool(name="tp_pool", bufs=1, space="PSUM"))
        t1_pool = ctx.enter_context(tc.tile_pool(name="t1_pool", bufs=10))
        exp_pool = ctx.enter_context(tc.tile_pool(name="exp_pool", bufs=22))
        oev_pool = ctx.enter_context(tc.tile_pool(name="oev_pool", bufs=6))
        yn_pool = ctx.enter_context(tc.tile_pool(name="yn_pool", bufs=4))
        rc_pool = ctx.enter_context(tc.tile_pool(name="rc_pool", bufs=4))

        # ---- Fused per-chunk pipeline: project chunk j, then attention for
        # chunk j (legal because causality means queries in chunk j only
        # attend to keys/values t <= chunk j), then its o-projection.
        # This overlaps PE-heavy projections of chunk j+1 with the
        # DVE/ACT-heavy softmax chain of chunk j. ----
        def project_chunk(j):
            t0 = j * CH
            xts = []
            for cb in range(NCB):
                xt = xT_pool.tile([P, CH], F16, name=f"xt_{j}_{cb}", tag="xt")
                nc.sync.dma_start(xt[:], xT_ap[cb * P : (cb + 1) * P, t0 : t0 + CH])
                xts.append(xt)
            for qh in range(NQH):
                ps = ps_pool.tile([P, CH], F32, name=f"psq_{j}_{qh}", tag="ps")
                for cb in range(NCB):
                    nc.tensor.matmul(
                        ps[:],
                        lhsT=qwT_sb[:, cb, qh * P : (qh + 1) * P],
                        rhs=xts[cb][:],
                        start=(cb == 0),
                        stop=(cb == NCB - 1),
                    )
                nc.vector.tensor_copy(QT_sb[:, qh, t0 : t0 + CH], ps[:])
            psk = ps_pool.tile([P, CH], F32, name=f"psk_{j}", tag="ps")
            for cb in range(NCB):
                nc.tensor.matmul(
                    psk[:],
                    lhsT=kwT_sb[:, cb, :],
                    rhs=xts[cb][:],
                    start=(cb == 0),
                    stop=(cb == NCB - 1),
                )
            nc.vector.tensor_copy(KT_sb[:, t0 : t0 + CH], psk[:])
            for tb in range(CH // P):
                gtb = j * (CH // P) + tb
                psv = ps_pool.tile([P, P], F32, name=f"psv_{j}_{tb}", tag="ps")
                for cb in range(NCB):
                    nc.tensor.matmul(
                        psv[:],
                        lhsT=xts[cb][:, tb * P : (tb + 1) * P],
                        rhs=vwT_sb[:, cb, :],
                        start=(cb == 0),
                        stop=(cb == NCB - 1),
                    )
                nc.vector.tensor_copy(Vaug_sb[:, gtb, 0:128], psv[:])

        def oproj_tblock(tb):
            for nch in range(C // CH):
                pso = ps_pool.tile([P, CH], F32, name=f"pso_{tb}_{nch}", tag="ps")
                for hb in range(NQH):
                    nc.tensor.matmul(
                        pso[:],
                        lhsT=yT_sb[:, hb, tb * P : (tb + 1) * P],
                        rhs=owT_sb[:, hb, nch * CH : (nch + 1) * CH],
                        start=(hb == 0),
                        stop=(hb == NQH - 1),
                    )
                ot = oev_pool.tile([P, CH], F16, name=f"ot_{tb}_{nch}", tag="ot")
                if (tb + nch) % 2 == 0:
                    nc.scalar.copy(ot[:], pso[:])
                else:
                    nc.vector.tensor_copy(ot[:], pso[:])
                nc.sync.dma_start(
                    out_ap[tb * P : (tb + 1) * P, nch * CH : (nch + 1) * CH], ot[:]
                )

        def attention_chunk(j):
            q0 = j * CH
            nkb = 4 * j + 4
            for h in range(NQH):
                ets = []
                for kb in range(nkb):
                    oi = kb - 4 * j
                    # q-columns below oi*P are fully causal-masked; skip them
                    qoff = oi * P if oi > 0 else 0
                    pss = ps_pool.tile([P, CH], F32, name=f"pss_{h}_{j}_{kb}", tag="ps")
                    nc.tensor.matmul(
                        pss[:, qoff:],
                        lhsT=KT_sb[:, kb * P : (kb + 1) * P],
                        rhs=QT_sb[:, h, q0 + qoff : q0 + CH],
                        start=True,
                        stop=True,
                    )
                    t1 = t1_pool.tile([P, CH], F32, name=f"t1_{h}_{j}_{kb}", tag="t1")
                    if oi >= 0:
                        nc.vector.tensor_add(
                            t1[:, qoff:], pss[:, qoff:], mcb_sb[:, h, oi, qoff:]
                        )
                    else:
                        nc.vector.tensor_add(t1[:], pss[:], mcl_sb[:, h, :])
                    et = exp_pool.tile([P, CH], F16, name=f"et_{h}_{j}_{kb}", tag="et")
                    oidx = oi + 12
                    nc.scalar.activation(
                        et[:, qoff:],
                        t1[:, qoff:],
                        EXP,
                        bias=bias_sb[:, h, oidx : oidx + 1],
                        scale=SCALE,
                    )
                    ets.append(et)
                for qb in range(CH // P):
                    gqb = j * (CH // P) + qb
                    yps = yps_pool.tile([P, 132], F32, name=f"yps_{h}_{gqb}", tag="yps")
                    for kb in range(gqb + 1):
                        nc.tensor.matmul(
                            yps[:, 0:129],
                            lhsT=ets[kb][:, qb * P : (qb + 1) * P],
                            rhs=Vaug_sb[:, kb, 0:129],
                            start=(kb == 0),
                            stop=(kb == gqb),
                        )
                    recip = rc_pool.tile([P, 1], F32, name=f"rc_{h}_{gqb}", tag="rc")
                    nc.vector.reciprocal(recip[:], yps[:, 128:129])
                    yn = yn_pool.tile([P, P], F16, name=f"yn_{h}_{gqb}", tag="yn")
                    nc.vector.tensor_scalar_mul(yn[:], yps[:, 0:128], recip[:])
                    tp = tp_pool.tile([P, P], F16, name=f"tp_{h}_{gqb}", tag="tp")
                    nc.tensor.transpose(tp[:], yn[:], id_sb[:])
                    nc.vector.tensor_copy(yT_sb[:, h, gqb * P : (gqb + 1) * P], tp[:])

        for j in range(NCHK):
            project_chunk(j)
            if j == 0:
                # constants first needed by attention/o-projection; issued
                # after chunk-0's projection DMAs so those aren't delayed
                nc.sync.dma_start(owT_sb[:], owT_ap[:])
                nc.sync.dma_start(mcl_sb[:], mcl_ap[:])
                nc.sync.dma_start(mcb_sb[:], mcb_ap[:])
                nc.sync.dma_start(bias_sb[:], bias_ap[:])
                nc.sync.dma_start(id_sb[:], id_ap[:])
            attention_chunk(j)
            for tb in range(j * (CH // P), (j + 1) * (CH // P)):
                oproj_tblock(tb)

    nc.compile()
    return nc


def make_in_maps(x, q_w, k_w, v_w, o_w):
    """Host-side sharding/preprocessing -> per-core input dicts."""
    slopes = _alibi_slopes(H)
    x_bf = np.asarray(x, dtype=NP_F16)

    ident = np.eye(P, dtype=NP_F16)

    pi = np.arange(P, dtype=np.float32)[:, None]
    mj = np.arange(CH, dtype=np.float32)[None, :]

    in_maps = []
    for c in range(8):
        b, g = c // 4, c % 4
        qsl = slice(4 * g * P, (4 * g + 4) * P)
        ksl = slice(g * P, (g + 1) * P)

        qwT = np.ascontiguousarray(
            np.asarray(q_w[qsl].T, dtype=NP_F16).reshape(NCB, P, NQH * P).transpose(1, 0, 2)
        )
        kwT = np.ascontiguousarray(
            np.asarray(k_w[ksl].T, dtype=NP_F16).reshape(NCB, P, D).transpose(1, 0, 2)
        )
        vwT = np.ascontiguousarray(
            np.asarray(v_w[ksl].T, dtype=NP_F16).reshape(NCB, P, D).transpose(1, 0, 2)
        )
        owT = np.ascontiguousarray(
            np.asarray(o_w[:, qsl].T, dtype=NP_F16).reshape(NQH, P, C).transpose(1, 0, 2)
        )

        mcl = np.empty((P, NQH, CH), dtype=np.float32)
        mcb = np.empty((P, NQH, 4, CH), dtype=np.float32)
        bias = np.empty((P, NQH, 16), dtype=np.float32)
        for h in range(NQH):
            sl = np.float32(slopes[4 * g + h])
            mcl[:, h, :] = (sl / np.float32(SCALE)) * (pi - mj)
            for oi in range(4):
                mcb[:, h, oi, :] = np.where(
                    oi * P + pi - mj > 0.0, np.float32(MASK_NEG), mcl[:, h, :]
                )
            for oidx in range(16):
                bias[:, h, oidx] = sl * np.float32(P * (oidx - 12))

        in_maps.append(
            dict(
                xT=np.ascontiguousarray(x_bf[b].T),
                qwT=qwT,
                kwT=kwT,
                vwT=vwT,
                owT=owT,
                mcl=mcl,
                mcb=mcb,
                bias=bias,
                ident=ident,
            )
        )
    return in_maps


def gather_output(results):
    out = np.zeros((B, T, C), dtype=np.float32)
    for c in range(8):
        out[c // 4] += results[c]["out_p"].astype(np.float32)
    return out


_NC_CACHE = {}


def get_program():
    if "nc" not in _NC_CACHE:
        _NC_CACHE["nc"] = build_program()
    return _NC_CACHE["nc"]


def kernel(x, q_w, k_w, v_w, o_w):
    from concourse.bass_utils import run_bass_kernel_spmd

    nc = get_program()
    in_maps = make_in_maps(x, q_w, k_w, v_w, o_w)
    res = run_bass_kernel_spmd(nc, in_maps, list(range(8)))
    return gather_output(res.results)
